# revision 2
# baseline (speedup 1.0000x reference)
"""Trainium2 Bass kernel for a 2-layer GraphSAGE (segment-mean aggregation).

8 cores SPMD; nodes sharded by id; edges partitioned by destination so each
core's scatter-mean is local. Per bin (<=32 consecutive nodes, <=512 edges)
the device gathers x[src] rows (indirect DMA, one 128-row gather per edge
tile), builds a recip-scaled one-hot on DVE, and a TensorE matmul
accumulates feature-major segment means into PSUM. Per 4 bins, two more
matmuls apply W_l/W_r and an epilogue adds bias (+relu). The halo exchange
between the two layers is a host-side all-gather of the node-major h shards
(two SPMD launches of per-layer programs).
"""

import sys
from contextlib import ExitStack

import numpy as np

try:
    import concourse.bass as bass
except ImportError:  # pragma: no cover
    sys.path.insert(0, "/opt/trn_rl_repo")
    import concourse.bass as bass

import concourse.bacc as bacc
import concourse.mybir as mybir
import concourse.tile as tile
from concourse.bass_utils import run_bass_kernel_spmd
from concourse.masks import make_identity

N = 50000
E = 800000
D = 128
NC = 8
T = 4
SLOTS_PER_BIN = T * 128
NPB = 32
GROUP = 4
BIN_ROUND = 8
OWN_CB = 4

F32 = mybir.dt.float32
I32 = mybir.dt.int32


def build_metadata(edge_index, n_nodes=N, n_cores=NC):
    src = np.asarray(edge_index[0], dtype=np.int64)
    dst = np.asarray(edge_index[1], dtype=np.int64)
    nsh = n_nodes // n_cores
    deg = np.bincount(dst, minlength=n_nodes)
    assert deg.max() <= SLOTS_PER_BIN
    recip = np.zeros(n_nodes, np.float32)
    nz = deg > 0
    recip[nz] = (1.0 / deg[nz]).astype(np.float32)

    order = np.argsort(dst, kind="stable")
    src_s = src[order]
    indptr = np.zeros(n_nodes + 1, np.int64)
    indptr[1:] = np.cumsum(deg)

    core_bins = []
    for c in range(n_cores):
        lo, hi = c * nsh, (c + 1) * nsh
        bins = []
        i = lo
        while i < hi:
            start = i
            s = 0
            while i < hi and (i - start) < NPB and s + deg[i] <= SLOTS_PER_BIN:
                s += deg[i]
                i += 1
            bins.append((start, i - start))
        core_bins.append(bins)

    B = max(len(b) for b in core_bins)
    B = -(-B // BIN_ROUND) * BIN_ROUND
    NSLOT = B * NPB
    OWN_C = NSLOT // 128
    NBATCH = B // BIN_ROUND
    OWN_CHUNKS = -(-OWN_C // OWN_CB)

    C = B * T
    gidx1 = np.zeros((n_cores, 128, C), np.int32)
    gidx2 = np.zeros((n_cores, 128, C), np.int32)
    seg = np.zeros((n_cores, 128, C), np.float32)
    rcp = np.zeros((n_cores, 128, C), np.float32)
    ownidx = np.zeros((n_cores, 128, OWN_C), np.int32)
    node_pos = np.full(n_nodes, -1, np.int64)

    for c in range(n_cores):
        for b, (nlo, nn) in enumerate(core_bins[c]):
            base = b * NPB
            nodes = np.arange(nlo, nlo + nn)
            slots = base + np.arange(nn)
            node_pos[nodes] = c * NSLOT + slots
            ownidx[c, slots % 128, slots // 128] = nodes
            degs = deg[nodes]
            ne = int(degs.sum())
            if ne == 0:
                continue
            s = np.arange(ne)
            q = np.repeat(np.arange(nn), degs)
            e0 = indptr[nlo]
            t_, p_ = s // 128, s % 128
            col = b * T + t_
            gidx1[c, p_, col] = src_s[e0:e0 + ne]
            seg[c, p_, col] = q
            rcp[c, p_, col] = np.repeat(recip[nodes], degs)

    assert np.all(node_pos >= 0)
    for c in range(n_cores):
        g2 = node_pos[gidx1[c]].astype(np.int32)
        g2[rcp[c] == 0.0] = 0
        gidx2[c] = g2

    def batched(a, w):
        nb = a.shape[-1] // w
        return np.ascontiguousarray(
            a.reshape(a.shape[0], 128, nb, w).transpose(0, 2, 1, 3))

    bw = BIN_ROUND * T
    md = dict(B=B, C=C, NSLOT=NSLOT, OWN_C=OWN_C, NBATCH=NBATCH,
              OWN_CHUNKS=OWN_CHUNKS, node_pos=node_pos,
              g1=batched(gidx1, bw), g2=batched(gidx2, bw),
              sg=batched(seg, bw), rc=batched(rcp, bw))
    pad = OWN_CHUNKS * OWN_CB - OWN_C
    if pad:
        ownidx = np.concatenate(
            [ownidx, np.zeros((n_cores, 128, pad), np.int32)], axis=-1)
    md["own"] = batched(ownidx, OWN_CB)
    md["iota"] = np.tile(np.arange(NPB, dtype=np.float32), (128, 1))
    return md


def build_layer_program(layer_no, n_nodes, B, n_cores=NC):
    NSLOT = B * NPB
    OWN_C = NSLOT // 128
    NBATCH = B // BIN_ROUND
    OWN_CHUNKS = -(-OWN_C // OWN_CB)
    NGROUP = B // GROUP
    bw = BIN_ROUND * T

    nc = bacc.Bacc("TRN2", target_bir_lowering=False, debug=False,
                   num_devices=n_cores)

    if layer_no == 1:
        tbl_ext = nc.dram_tensor("x", [n_nodes, D], F32, kind="ExternalInput")
        g_ext = nc.dram_tensor("g1", [NBATCH, 128, bw], I32,
                               kind="ExternalInput")
        own_ext = nc.dram_tensor("own", [OWN_CHUNKS, 128, OWN_CB], I32,
                                 kind="ExternalInput")
        out_ext = nc.dram_tensor("h_shard", [NSLOT, D], F32,
                                 kind="ExternalOutput")
        wl_name, wr_name, b_name = "W1l", "W1r", "b1"
    else:
        tbl_ext = nc.dram_tensor("hfull", [n_cores * NSLOT, D], F32,
                                 kind="ExternalInput")
        hown_ext = nc.dram_tensor("hown", [NSLOT, D], F32,
                                  kind="ExternalInput")
        g_ext = nc.dram_tensor("g2", [NBATCH, 128, bw], I32,
                               kind="ExternalInput")
        out_ext = nc.dram_tensor("outT", [NGROUP, D, GROUP * NPB], F32,
                                 kind="ExternalOutput")
        wl_name, wr_name, b_name = "W2l", "W2r", "b2"

    sg_ext = nc.dram_tensor("sg", [NBATCH, 128, bw], F32, kind="ExternalInput")
    rc_ext = nc.dram_tensor("rc", [NBATCH, 128, bw], F32, kind="ExternalInput")
    iota_ext = nc.dram_tensor("iota", [128, NPB], F32, kind="ExternalInput")
    wl_ext = nc.dram_tensor(wl_name, [D, D], F32, kind="ExternalInput")
    wr_ext = nc.dram_tensor(wr_name, [D, D], F32, kind="ExternalInput")
    bias_ext = nc.dram_tensor(b_name, [D, 1], F32, kind="ExternalInput")

    with tile.TileContext(nc) as tc, ExitStack() as ctx:
        const = ctx.enter_context(tc.tile_pool(name="const", bufs=1))
        gpool = ctx.enter_context(tc.tile_pool(name="gather", bufs=3))
        mpool = ctx.enter_context(tc.tile_pool(name="meta", bufs=4))
        ohpool = ctx.enter_context(tc.tile_pool(name="oh", bufs=4))
        stpool = ctx.enter_context(tc.tile_pool(name="stage", bufs=4))
        pseg = ctx.enter_context(tc.tile_pool(name="pseg", bufs=2, space="PSUM"))
        pw = ctx.enter_context(tc.tile_pool(name="pw", bufs=2, space="PSUM"))
        pt = ctx.enter_context(tc.tile_pool(name="pt", bufs=2, space="PSUM"))

        Wl = const.tile([D, D], F32, name="Wl")
        nc.sync.dma_start(Wl[:], wl_ext[:, :])
        Wr = const.tile([D, D], F32, name="Wr")
        nc.sync.dma_start(Wr[:], wr_ext[:, :])
        bias = const.tile([D, 1], F32, name="bias")
        nc.sync.dma_start(bias[:], bias_ext[:, :])
        iota_sb = const.tile([128, NPB], F32, name="iota_sb")
        nc.sync.dma_start(iota_sb[:], iota_ext[:, :])
        ident = const.tile([128, 128], F32, name="ident")
        make_identity(nc, ident[:])

        ownT = const.tile([128, NSLOT], F32, name="ownT")

        def iota_rep(k):
            ap = iota_sb[:, :]
            return bass.AP(ap.tensor, ap.offset,
                           [[NPB, 128], [0, k], [1, NPB]])

        # ---- own-feature transpose path: ownT = (own rows)^T
        if layer_no == 1:
            for chk in range(OWN_CHUNKS):
                oi = mpool.tile([128, OWN_CB], I32, tag="oi", name="oi")
                nc.sync.dma_start(oi[:], own_ext[chk])
                ob = gpool.tile([128, OWN_CB * 128], F32, tag="ob", name="ob")
                for j in range(OWN_CB):
                    nc.gpsimd.indirect_dma_start(
                        out=ob[:, j * 128:(j + 1) * 128], out_offset=None,
                        in_=tbl_ext[:, :],
                        in_offset=bass.IndirectOffsetOnAxis(
                            ap=oi[:, j:j + 1], axis=0))
                for j in range(OWN_CB):
                    col = chk * OWN_CB + j
                    if col >= OWN_C:
                        break
                    tp = pt.tile([128, 128], F32, tag="tp", name="tp")
                    nc.tensor.transpose(tp[:], ob[:, j * 128:(j + 1) * 128],
                                        ident[:])
                    nc.vector.tensor_copy(ownT[:, col * 128:(col + 1) * 128],
                                          tp[:])
        else:
            for g in range(OWN_C):
                ho = gpool.tile([128, 128], F32, tag="ho", name="ho")
                nc.sync.dma_start(ho[:], hown_ext[g * 128:(g + 1) * 128, :])
                tp = pt.tile([128, 128], F32, tag="tp", name="tp")
                nc.tensor.transpose(tp[:], ho[:], ident[:])
                nc.vector.tensor_copy(ownT[:, g * 128:(g + 1) * 128], tp[:])

        # ---- main path
        for eb in range(NBATCH):
            gi = mpool.tile([128, bw], I32, tag="gi", name="gi")
            nc.sync.dma_start(gi[:], g_ext[eb])
            gb = gpool.tile([128, bw * 128], F32, tag="gb", name="gb")
            for j in range(bw):
                nc.gpsimd.indirect_dma_start(
                    out=gb[:, j * 128:(j + 1) * 128], out_offset=None,
                    in_=tbl_ext[:, :],
                    in_offset=bass.IndirectOffsetOnAxis(ap=gi[:, j:j + 1],
                                                        axis=0))
            sgt = mpool.tile([128, bw], F32, tag="sgt", name="sgt")
            nc.sync.dma_start(sgt[:], sg_ext[eb])
            rct = mpool.tile([128, bw], F32, tag="rct", name="rct")
            nc.sync.dma_start(rct[:], rc_ext[eb])
            mt = None
            for bi in range(BIN_ROUND):
                b = eb * BIN_ROUND + bi
                oh = ohpool.tile([128, T * NPB], F32, tag="oh", name="oh")
                oh3 = oh[:].rearrange("p (t q) -> p t q", q=NPB)
                nc.vector.tensor_tensor(
                    out=oh3,
                    in0=sgt[:, bi * T:(bi + 1) * T].to_broadcast(
                        [128, T, NPB]),
                    in1=iota_rep(T), op=mybir.AluOpType.is_equal)
                nc.vector.tensor_tensor(
                    out=oh3, in0=oh3,
                    in1=rct[:, bi * T:(bi + 1) * T].to_broadcast(
                        [128, T, NPB]),
                    op=mybir.AluOpType.mult)
                ps = pseg.tile([128, NPB], F32, tag="ps", name="ps")
                for t in range(T):
                    cx = (bi * T + t) * 128
                    nc.tensor.matmul(ps[:], lhsT=gb[:, cx:cx + 128],
                                     rhs=oh[:, t * NPB:(t + 1) * NPB],
                                     start=(t == 0), stop=(t == T - 1))
                if b % GROUP == 0:
                    mt = stpool.tile([128, GROUP * NPB], F32, tag="mt",
                                     name="mt")
                qq = (b % GROUP) * NPB
                nc.vector.tensor_copy(mt[:, qq:qq + NPB], ps[:])
                if b % GROUP == GROUP - 1:
                    g = b // GROUP
                    wp = pw.tile([128, GROUP * NPB], F32, tag="wp", name="wp")
                    nc.tensor.matmul(wp[:], lhsT=Wl[:], rhs=mt[:],
                                     start=True, stop=False)
                    nc.tensor.matmul(wp[:], lhsT=Wr[:],
                                     rhs=ownT[:, g * 128:(g + 1) * 128],
                                     start=False, stop=True)
                    if layer_no == 1:
                        hT = stpool.tile([128, 128], F32, tag="hT", name="hT")
                        nc.scalar.activation(
                            out=hT[:], in_=wp[:],
                            func=mybir.ActivationFunctionType.Relu,
                            bias=bias[:, :1])
                        tp = pt.tile([128, 128], F32, tag="tp", name="tp2")
                        nc.tensor.transpose(tp[:], hT[:], ident[:])
                        hs = stpool.tile([128, 128], F32, tag="hs", name="hs")
                        nc.vector.tensor_copy(hs[:], tp[:])
                        nc.sync.dma_start(out_ext[g * 128:(g + 1) * 128, :],
                                          hs[:])
                    else:
                        osb = stpool.tile([128, GROUP * NPB], F32, tag="os",
                                          name="osb")
                        nc.vector.tensor_scalar_add(osb[:], wp[:],
                                                    bias[:, :1])
                        nc.sync.dma_start(out_ext[g], osb[:])

    nc.compile()
    return nc


_CACHE = {}


def kernel(**inputs) -> np.ndarray:
    md = build_metadata(inputs["edge_index"])
    B, NSLOT = md["B"], md["NSLOT"]
    if ("p1", B) not in _CACHE:
        _CACHE[("p1", B)] = build_layer_program(1, N, B)
        _CACHE[("p2", B)] = build_layer_program(2, N, B)
    p1, p2 = _CACHE[("p1", B)], _CACHE[("p2", B)]

    x = np.ascontiguousarray(np.asarray(inputs["x"], np.float32))
    W = {k: np.ascontiguousarray(np.asarray(inputs[k], np.float32))
         for k in ("W1l", "W1r", "W2l", "W2r")}
    b1 = np.asarray(inputs["b1"], np.float32).reshape(D, 1)
    b2 = np.asarray(inputs["b2"], np.float32).reshape(D, 1)

    maps1 = [dict(x=x, g1=np.ascontiguousarray(md["g1"][c]),
                  own=np.ascontiguousarray(md["own"][c]),
                  sg=np.ascontiguousarray(md["sg"][c]),
                  rc=np.ascontiguousarray(md["rc"][c]),
                  iota=md["iota"], W1l=W["W1l"], W1r=W["W1r"], b1=b1)
             for c in range(NC)]
    r1 = run_bass_kernel_spmd(p1, maps1, core_ids=list(range(NC)))
    h_full = np.ascontiguousarray(
        np.concatenate([np.asarray(r1.results[c]["h_shard"])
                        for c in range(NC)], axis=0))

    maps2 = [dict(hfull=h_full,
                  hown=h_full[c * NSLOT:(c + 1) * NSLOT],
                  g2=np.ascontiguousarray(md["g2"][c]),
                  sg=np.ascontiguousarray(md["sg"][c]),
                  rc=np.ascontiguousarray(md["rc"][c]),
                  iota=md["iota"], W2l=W["W2l"], W2r=W["W2r"], b2=b2)
             for c in range(NC)]
    r2 = run_bass_kernel_spmd(p2, maps2, core_ids=list(range(NC)))

    outs = []
    for c in range(NC):
        o = np.asarray(r2.results[c]["outT"])
        outs.append(o.transpose(0, 2, 1).reshape(-1, D))
    full = np.concatenate(outs, axis=0)
    return np.ascontiguousarray(full[md["node_pos"]])


if __name__ == "__main__":
    import reference
    inputs = {k: np.asarray(v) for k, v in reference.setup_inputs().items()}
    out = kernel(**inputs)
    print(out.shape, out.dtype)
